# revision 1
# baseline (speedup 1.0000x reference)
"""Grouped-experts SwiGLU kernel for 8 Trainium2 NeuronCores.

Problem: x[E,T,D], w1[E,D,H], w2[E,H,D], w3[E,D,H] with E=8, T=1024,
D=1024, H=2048.  out_e = (silu(x_e @ w1_e) * (x_e @ w3_e)) @ w2_e.

Sharding: expert-parallel, one expert per NeuronCore (E == n_cores == 8).
Each core runs an identical Bass program on its expert's slices; no
collectives are needed and the full output is just the stack of the
per-core outputs.

Per-core schedule (all matmuls in float32r => full PE rate at N=512):
  0. Load x natural [T,D] tile-by-tile, transpose 128x128 blocks on the
     TensorEngine (identity transpose) into resident xT (partition = D).
  1. Stage A: for each of the 16 H-tiles, stream w1/w3 slices, compute
     gT/upT = w^T @ x^T in PSUM accumulating over the 8 D-chunks,
     silu on ScalarE, multiply on VectorE -> resident hT (partition = H).
  2. Stage B: for each 512-wide D-chunk, stream w2 slices, compute
     out = h @ w2 accumulating over the 16 H-chunks; PSUM evacuated via
     ScalarE/VectorE copies and DMAed out in natural [T,D] layout.
"""

import sys

if "/opt/trn_rl_repo" not in sys.path:
    sys.path.insert(0, "/opt/trn_rl_repo")

import numpy as np

E, T, D, H = 8, 1024, 1024, 2048
P = 128
NT, ND, NH = T // P, D // P, H // P
TC = 512  # stage-A moving (token) chunk
DC = 512  # stage-B moving (dim) chunk
NTC, NDC = T // TC, D // DC


def build_program(reps: int = 1):
    """Build the per-core Bass program. reps>1 repeats the whole compute
    body (for wall-clock slope timing); the result is identical."""
    import concourse.bacc as bacc
    import concourse.mybir as mybir
    from concourse import tile, masks

    f32 = mybir.dt.float32
    f32r = mybir.dt.float32r
    SILU = mybir.ActivationFunctionType.Silu

    nc = bacc.Bacc("TRN2", target_bir_lowering=False, debug=False)
    x_d = nc.declare_dram_parameter("x", [T, D], f32, isOutput=False)
    w1_d = nc.declare_dram_parameter("w1", [D, H], f32, isOutput=False)
    w2_d = nc.declare_dram_parameter("w2", [H, D], f32, isOutput=False)
    w3_d = nc.declare_dram_parameter("w3", [D, H], f32, isOutput=False)
    out_d = nc.declare_dram_parameter("out", [T, D], f32, isOutput=True)

    # DRAM views with the partition dim innermost of the leading axis.
    w1_v = w1_d[:].rearrange("(dd p) hh -> p dd hh", p=P)  # [128, ND, H]
    w3_v = w3_d[:].rearrange("(dd p) hh -> p dd hh", p=P)
    w2_v = w2_d[:].rearrange("(hh p) dd -> p hh dd", p=P)  # [128, NH, D]

    with tile.TileContext(nc) as tc:
        with (
            tc.tile_pool(name="const", bufs=1) as const_pool,
            tc.tile_pool(name="xT", bufs=1) as xT_pool,
            tc.tile_pool(name="hT", bufs=1) as hT_pool,
            tc.tile_pool(name="xs", bufs=2) as xs_pool,
            tc.tile_pool(name="wA", bufs=3) as wA_pool,
            tc.tile_pool(name="sg", bufs=3) as sg_pool,
            tc.tile_pool(name="wB", bufs=20) as wB_pool,
            tc.tile_pool(name="ob", bufs=4) as ob_pool,
            tc.tile_pool(name="ps", bufs=8, space="PSUM") as ps_pool,
        ):
            ident = const_pool.tile([P, P], f32, name="ident", tag="ident")
            masks.make_identity(nc, ident[:])

            for rep in range(reps):
                # ---- Phase 0: load x and transpose into xT (partition=D) --
                # float32r tiles: producers round at write time, as the BIR
                # verifier requires for fp32r matmul inputs.
                xT = [
                    xT_pool.tile([P, T], f32r, name=f"xT{dd}", tag=f"xT{dd}")
                    for dd in range(ND)
                ]
                for tt in range(NT):
                    xs = xs_pool.tile([P, D], f32, name="xs", tag="xs")
                    nc.sync.dma_start(out=xs[:], in_=x_d[tt * P : (tt + 1) * P, :])
                    for dd in range(ND):
                        pt = ps_pool.tile([P, P], f32, name="pt", tag="ps")
                        nc.tensor.transpose(
                            pt[:], xs[:, dd * P : (dd + 1) * P], ident[:]
                        )
                        dst = xT[dd][:, tt * P : (tt + 1) * P]
                        if dd % 2 == 0:
                            nc.vector.tensor_copy(dst, pt[:])
                        else:
                            nc.scalar.copy(dst, pt[:])

                # ---- Stage A: hT = silu(w1^T x^T) * (w3^T x^T) ------------
                hT = [
                    hT_pool.tile([P, T], f32r, name=f"hT{hh}", tag=f"hT{hh}")
                    for hh in range(NH)
                ]
                for hh in range(NH):
                    w1s = wA_pool.tile([P, ND, P], f32r, name="w1s", tag="w1s")
                    nc.sync.dma_start(
                        out=w1s[:], in_=w1_v[:, :, hh * P : (hh + 1) * P].bitcast(f32r)
                    )
                    w3s = wA_pool.tile([P, ND, P], f32r, name="w3s", tag="w3s")
                    nc.sync.dma_start(
                        out=w3s[:], in_=w3_v[:, :, hh * P : (hh + 1) * P].bitcast(f32r)
                    )
                    for c in range(NTC):
                        tok = slice(c * TC, (c + 1) * TC)
                        g_ps = ps_pool.tile([P, TC], f32, name="g_ps", tag="ps")
                        u_ps = ps_pool.tile([P, TC], f32, name="u_ps", tag="ps")
                        for dd in range(ND):
                            nc.tensor.matmul(
                                g_ps[:],
                                w1s[:, dd, :],
                                xT[dd][:, tok],
                                start=(dd == 0),
                                stop=(dd == ND - 1),
                            )
                        for dd in range(ND):
                            nc.tensor.matmul(
                                u_ps[:],
                                w3s[:, dd, :],
                                xT[dd][:, tok],
                                start=(dd == 0),
                                stop=(dd == ND - 1),
                            )
                        sg = sg_pool.tile([P, TC], f32, name="sg", tag="sg")
                        nc.scalar.activation(sg[:], g_ps[:], SILU)
                        nc.vector.tensor_mul(hT[hh][:, tok], sg[:], u_ps[:])

                # ---- Stage B: out = h @ w2 --------------------------------
                for dc in range(NDC):
                    dcs = slice(dc * DC, (dc + 1) * DC)
                    w2s = []
                    for hh in range(NH):
                        w2t = wB_pool.tile([P, DC], f32r, name="w2t", tag="w2t")
                        nc.sync.dma_start(
                            out=w2t[:], in_=w2_v[:, hh, dcs].bitcast(f32r)
                        )
                        w2s.append(w2t)
                    for t in range(NT):
                        o_ps = ps_pool.tile([P, DC], f32, name="o_ps", tag="ps")
                        for hh in range(NH):
                            nc.tensor.matmul(
                                o_ps[:],
                                hT[hh][:, t * P : (t + 1) * P],
                                w2s[hh][:],
                                start=(hh == 0),
                                stop=(hh == NH - 1),
                            )
                        ob = ob_pool.tile([P, DC], f32, name="ob", tag="ob")
                        if t % 2 == 0:
                            nc.vector.tensor_copy(ob[:], o_ps[:])
                        else:
                            nc.scalar.copy(ob[:], o_ps[:])
                        nc.scalar.dma_start(
                            out=out_d[t * P : (t + 1) * P, dcs], in_=ob[:]
                        )

    nc.compile()
    return nc


_program_cache = {}


def _get_program(reps: int = 1):
    if reps not in _program_cache:
        _program_cache[reps] = build_program(reps)
    return _program_cache[reps]


def kernel(x, w1, w2, w3):
    from concourse.bass_utils import run_bass_kernel_spmd

    x = np.ascontiguousarray(np.asarray(x, dtype=np.float32))
    w1 = np.ascontiguousarray(np.asarray(w1, dtype=np.float32))
    w2 = np.ascontiguousarray(np.asarray(w2, dtype=np.float32))
    w3 = np.ascontiguousarray(np.asarray(w3, dtype=np.float32))

    nc = _get_program()
    in_maps = [
        {"x": x[e], "w1": w1[e], "w2": w2[e], "w3": w3[e]} for e in range(E)
    ]
    res = run_bass_kernel_spmd(nc, in_maps, list(range(E)))
    out = np.stack([res.results[e]["out"] for e in range(E)], axis=0)
    return out.astype(np.float32)



# revision 3
# speedup vs baseline: 1.1883x; 1.1883x over previous
"""Grouped-experts SwiGLU kernel for 8 Trainium2 NeuronCores.

Problem: x[E,T,D], w1[E,D,H], w2[E,H,D], w3[E,D,H] with E=8, T=1024,
D=1024, H=2048.  out_e = (silu(x_e @ w1_e) * (x_e @ w3_e)) @ w2_e.

Sharding: expert-parallel, one expert per NeuronCore (E == n_cores == 8).
Each core runs an identical Bass program on its expert's slices; no
collectives are needed and the full output is just the stack of the
per-core outputs.

All matmul inputs are staged host-side into bf16 (end-to-end rel err
~3.5e-3 vs the 2e-2 budget; PSUM accumulation stays fp32):
  xt  [D, T]        x transposed (partition dim = D, ready for the PE)
  w1r [H, ND*128]   w1 reordered so tile hh is one contiguous 256KB block
                    with layout [p=d%128, dd, h-col]
  w3r               same as w1r
  w2r [H, D]        natural (row block hh is the stage-B rhs)

Per-core schedule (bf16 matmuls run 1 col/cycle at N=512):
  warm-up: a few dummy matmuls so the HAM clock gate reaches 2.4 GHz
           before real work arrives.
  Stage A: for each of the 16 H-tiles, stream w1r/w3r slices (scalar HW
           DMA queue), compute gT/upT = w^T @ x^T in PSUM accumulating
           over the 8 D-chunks, silu on ScalarE, multiply on VectorE ->
           resident hT bf16 (partition = H).
  Stage B: w2 fully resident (prefetched on the sync queue behind xt),
           out = h @ w2 accumulating over the 16 H-chunks; PSUM
           evacuated via ScalarE/VectorE copies, DMA out in natural
           [T,D] fp32 layout.
"""

import sys

if "/opt/trn_rl_repo" not in sys.path:
    sys.path.insert(0, "/opt/trn_rl_repo")

import numpy as np
import ml_dtypes

E, T, D, H = 8, 1024, 1024, 2048
P = 128
NT, ND, NH = T // P, D // P, H // P
TC = 512  # stage-A moving (token) chunk
NTC = T // TC
BF16 = ml_dtypes.bfloat16


def build_program(reps: int = 1):
    """Build the per-core Bass program. reps>1 repeats the whole compute
    body (for wall-clock slope timing); the result is identical."""
    import concourse.bacc as bacc
    import concourse.mybir as mybir
    from concourse import tile

    f32 = mybir.dt.float32
    bf16 = mybir.dt.bfloat16
    SILU = mybir.ActivationFunctionType.Silu

    nc = bacc.Bacc("TRN2", target_bir_lowering=False, debug=False)
    xt_d = nc.declare_dram_parameter("xt", [D, T], bf16, isOutput=False)
    w1_d = nc.declare_dram_parameter("w1r", [H, ND * P], bf16, isOutput=False)
    w2_d = nc.declare_dram_parameter("w2r", [H, D], bf16, isOutput=False)
    w3_d = nc.declare_dram_parameter("w3r", [H, ND * P], bf16, isOutput=False)
    out_d = nc.declare_dram_parameter("out", [T, D], f32, isOutput=True)

    with tile.TileContext(nc) as tc:
        with (
            tc.tile_pool(name="warm", bufs=1) as warm_pool,
            tc.tile_pool(name="xT", bufs=1) as xT_pool,
            tc.tile_pool(name="hT", bufs=1) as hT_pool,
            tc.tile_pool(name="w2s", bufs=1) as w2_pool,
            tc.tile_pool(name="wA", bufs=8) as wA_pool,
            tc.tile_pool(name="sg", bufs=4) as sg_pool,
            tc.tile_pool(name="ob", bufs=4) as ob_pool,
            tc.tile_pool(name="ps", bufs=8, space="PSUM") as ps_pool,
        ):
            # Scratch operand for PE warm-up matmuls (contents irrelevant).
            wu = warm_pool.tile([P, 2 * P], bf16, name="wu", tag="wu")
            nc.gpsimd.memset(wu[:], 0.0)

            for rep in range(reps):
                # ---- input DMAs -----------------------------------------
                # sync queue: xt halves (stage A's critical path), then the
                # whole of w2 (consumed from ~2/3 into the kernel).
                xT = [
                    xT_pool.tile([P, T], bf16, name=f"xT{dd}", tag=f"xT{dd}")
                    for dd in range(ND)
                ]
                for c in range(NTC):
                    tok = slice(c * TC, (c + 1) * TC)
                    for dd in range(ND):
                        nc.sync.dma_start(
                            out=xT[dd][:, tok], in_=xt_d[dd * P : (dd + 1) * P, tok]
                        )
                w2s = []
                for hh in range(NH):
                    w2t = w2_pool.tile([P, D], bf16, name=f"w2t{hh}", tag=f"w2t{hh}")
                    nc.sync.dma_start(
                        out=w2t[:], in_=w2_d[hh * P : (hh + 1) * P, :]
                    )
                    w2s.append(w2t)

                # ---- PE warm-up: ~3us of dummy matmuls so the HAM clock
                # gate un-throttles before the first real matmul.
                if rep == 0:
                    wu_ps = ps_pool.tile([P, 2 * P], f32, name="wu_ps", tag="ps")
                    for _ in range(16):
                        nc.tensor.matmul(
                            wu_ps[:], wu[:, :P], wu[:], start=True, stop=True
                        )

                # ---- Stage A: hT = silu(w1^T x^T) * (w3^T x^T) ----------
                hT = [
                    hT_pool.tile([P, T], bf16, name=f"hT{hh}", tag=f"hT{hh}")
                    for hh in range(NH)
                ]
                for hh in range(NH):
                    w1s = wA_pool.tile([P, ND, P], bf16, name="w1s", tag="w1s")
                    nc.scalar.dma_start(
                        out=w1s[:],
                        in_=w1_d[hh * P : (hh + 1) * P, :].rearrange(
                            "p (dd c) -> p dd c", c=P
                        ),
                    )
                    w3s = wA_pool.tile([P, ND, P], bf16, name="w3s", tag="w3s")
                    nc.scalar.dma_start(
                        out=w3s[:],
                        in_=w3_d[hh * P : (hh + 1) * P, :].rearrange(
                            "p (dd c) -> p dd c", c=P
                        ),
                    )
                    for c in range(NTC):
                        tok = slice(c * TC, (c + 1) * TC)
                        g_ps = ps_pool.tile([P, TC], f32, name="g_ps", tag="ps")
                        u_ps = ps_pool.tile([P, TC], f32, name="u_ps", tag="ps")
                        for dd in range(ND):
                            nc.tensor.matmul(
                                g_ps[:],
                                w1s[:, dd, :],
                                xT[dd][:, tok],
                                start=(dd == 0),
                                stop=(dd == ND - 1),
                            )
                        for dd in range(ND):
                            nc.tensor.matmul(
                                u_ps[:],
                                w3s[:, dd, :],
                                xT[dd][:, tok],
                                start=(dd == 0),
                                stop=(dd == ND - 1),
                            )
                        sg = sg_pool.tile([P, TC], f32, name="sg", tag="sg")
                        nc.scalar.activation(sg[:], g_ps[:], SILU)
                        nc.vector.tensor_mul(hT[hh][:, tok], sg[:], u_ps[:])

                # ---- Stage B: out = h @ w2 ------------------------------
                for t in range(NT):
                    ob = ob_pool.tile([P, D], f32, name="ob", tag="ob")
                    for dc in range(2):
                        dcs = slice(dc * TC, (dc + 1) * TC)
                        o_ps = ps_pool.tile([P, TC], f32, name="o_ps", tag="ps")
                        for hh in range(NH):
                            nc.tensor.matmul(
                                o_ps[:],
                                hT[hh][:, t * P : (t + 1) * P],
                                w2s[hh][:, dcs],
                                start=(hh == 0),
                                stop=(hh == NH - 1),
                            )
                        if dc == 0:
                            nc.vector.tensor_copy(ob[:, dcs], o_ps[:])
                        else:
                            nc.scalar.copy(ob[:, dcs], o_ps[:])
                    nc.sync.dma_start(
                        out=out_d[t * P : (t + 1) * P, :], in_=ob[:]
                    )

    nc.compile()
    return nc


_program_cache = {}


def _get_program(reps: int = 1):
    if reps not in _program_cache:
        _program_cache[reps] = build_program(reps)
    return _program_cache[reps]


def stage_inputs(x_e, w1_e, w2_e, w3_e):
    """Host-side staging of one expert's inputs into the device layouts."""
    xt = np.ascontiguousarray(x_e.T).astype(BF16)
    w1r = (
        w1_e.reshape(ND, P, NH, P)
        .transpose(2, 1, 0, 3)
        .astype(BF16)
        .reshape(H, ND * P)
    )
    w3r = (
        w3_e.reshape(ND, P, NH, P)
        .transpose(2, 1, 0, 3)
        .astype(BF16)
        .reshape(H, ND * P)
    )
    w2r = w2_e.astype(BF16)
    return {"xt": xt, "w1r": w1r, "w2r": w2r, "w3r": w3r}


def kernel(x, w1, w2, w3):
    from concourse.bass_utils import run_bass_kernel_spmd

    x = np.asarray(x, dtype=np.float32)
    w1 = np.asarray(w1, dtype=np.float32)
    w2 = np.asarray(w2, dtype=np.float32)
    w3 = np.asarray(w3, dtype=np.float32)

    nc = _get_program()
    in_maps = [stage_inputs(x[e], w1[e], w2[e], w3[e]) for e in range(E)]
    res = run_bass_kernel_spmd(nc, in_maps, list(range(E)))
    out = np.stack([res.results[e]["out"] for e in range(E)], axis=0)
    return out.astype(np.float32)
